# revision 1
# baseline (speedup 1.0000x reference)
"""Cross-attention kernel for Trainium2, SPMD over 8 NeuronCores.

Reference computation (per batch b):
    x       = channel_img[b].reshape(C, N)          # [512, 1024], N = 32*32
    query   = tanh(Wq @ h[b] + bq)                  # [512]
    keysT   = tanh(Wk @ x + bk[:, None])            # [512, 1024]   (d, n)
    valsT   = tanh(Wv @ x + bv[:, None])            # [512, 1024]   (d, n)
    scores  = query @ keysT                         # [1024]
    w       = softmax(scores)
    out[b]  = valsT @ w                             # [512]

Sharding: data-parallel over batch, 8 batches per core, weights replicated.

f32r design (default): all matmul operands are float32r (fp32 storage,
reduced-precision PE mode, 1 cyc/row at moving dim >= 256 — same speed as
bf16 but ~16x more accurate on HW). Per batch:
  - keys/values projections: 64 matmuls N=512, bias+tanh fused on ScalarE
  - scores: M=1 matvec (8 matmuls N=512), exp + running sum fused on ScalarE
  - context: normalized w broadcast to all partitions via one K=1 matmul per
    half (lhsT = row of 1/sum(w)), then VectorE tensor_tensor_reduce does
    the weighted reduction valsT . w along the free dim — no PE transposes,
    no context matvec on PE.

bf16 fallback design keeps values in [n, d] orientation with a PE context
matvec and PE-based w transposes.
"""

import numpy as np
import ml_dtypes
from contextlib import ExitStack

import concourse.bass as bass
import concourse.tile as tile
from concourse import bacc, mybir
from concourse.bass import ds
from concourse.bass_utils import run_bass_kernel_spmd

P = 128          # SBUF partitions
G = 4            # 512 = G * P groups along the hidden dim
D = 512          # hidden size
N = 1024         # spatial positions (32*32)
NB = 8           # batches per core
NCORES = 8
BF = mybir.dt.bfloat16
F32 = mybir.dt.float32
F32R = mybir.dt.float32r
Tanh = mybir.ActivationFunctionType.Tanh
Exp = mybir.ActivationFunctionType.Exp
Copy = mybir.ActivationFunctionType.Copy
Mult = mybir.AluOpType.mult
Add = mybir.AluOpType.add

MODE = "f32r"    # default mode used by kernel()

_CACHED = {}


def _build_f32r_v1(repeat=1, img_internal=False,
                   bmm=4, bvec=2, bt=2, bimg=2, bkv=2, bsm=4):
    """HW-validated f32r design: values in [n, d] orientation, PE context
    matvec, w~ transposed via K=1/N=2 matmuls. ~141 us/rep steady-state."""
    nc = bacc.Bacc("TRN2", target_bir_lowering=False, debug=False,
                   num_devices=NCORES)

    img_kind = "Internal" if img_internal else "ExternalInput"
    img_ap = nc.dram_tensor("img", [NB, D, N], F32R, kind=img_kind).ap()
    ht_ap = nc.dram_tensor("hT", [P, G, NB], F32R, kind="ExternalInput").ap()
    wq_ap = nc.dram_tensor("wqT", [P, G, D], F32R, kind="ExternalInput").ap()
    wk_ap = nc.dram_tensor("wkT", [P, G, D], F32R, kind="ExternalInput").ap()
    wv_ap = nc.dram_tensor("wvT", [P, G, D], F32R, kind="ExternalInput").ap()
    bq_ap = nc.dram_tensor("bqT", [P, G], F32, kind="ExternalInput").ap()
    bk_ap = nc.dram_tensor("bkT", [P, G], F32, kind="ExternalInput").ap()
    bvb_ap = nc.dram_tensor("bvb", [P, D], F32, kind="ExternalInput").ap()
    one_ap = nc.dram_tensor("onec", [1, 2], F32R, kind="ExternalInput").ap()
    out_ap = nc.dram_tensor("out", [NB, D], F32, kind="ExternalOutput").ap()

    def mm(out, lhsT, rhs, start, stop):
        nc.tensor.matmul(out, lhsT=lhsT, rhs=rhs, start=start, stop=stop)

    with tile.TileContext(nc) as tc, ExitStack() as ctx:
        consts = ctx.enter_context(tc.tile_pool(name="consts", bufs=1))
        pimg32 = ctx.enter_context(tc.tile_pool(name="pimg32", bufs=bimg))
        pkeys = ctx.enter_context(tc.tile_pool(name="pkeys", bufs=bkv))
        pvals = ctx.enter_context(tc.tile_pool(name="pvals", bufs=bkv))
        psmall = ctx.enter_context(tc.tile_pool(name="psmall", bufs=bsm))
        ppmm = ctx.enter_context(tc.tile_pool(name="ppmm", bufs=bmm, space="PSUM"))
        ppvec = ctx.enter_context(tc.tile_pool(name="ppvec", bufs=bvec, space="PSUM"))
        ppt = ctx.enter_context(tc.tile_pool(name="ppt", bufs=bt, space="PSUM"))

        wq = consts.tile([P, G, D], F32R, tag="wq")
        nc.sync.dma_start(out=wq, in_=wq_ap)
        wk = consts.tile([P, G, D], F32R, tag="wk")
        nc.sync.dma_start(out=wk, in_=wk_ap)
        wv = consts.tile([P, G, D], F32R, tag="wv")
        nc.sync.dma_start(out=wv, in_=wv_ap)
        bq = consts.tile([P, G], F32, tag="bq")
        nc.sync.dma_start(out=bq, in_=bq_ap)
        bk = consts.tile([P, G], F32, tag="bk")
        nc.sync.dma_start(out=bk, in_=bk_ap)
        bvb = consts.tile([P, D], F32, tag="bvb")
        nc.sync.dma_start(out=bvb, in_=bvb_ap)
        ones = consts.tile([1, 2], F32R, tag="ones")
        nc.sync.dma_start(out=ones, in_=one_ap)
        ht = consts.tile([P, G, NB], F32R, tag="ht")
        nc.sync.dma_start(out=ht, in_=ht_ap)

        qt = consts.tile([P, G, NB], F32R, tag="qt")
        for dg in range(G):
            pq = ppt.tile([P, NB], F32, tag="t")
            for cg in range(G):
                mm(pq, wq[:, cg, ds(dg * P, P)], ht[:, cg, :],
                   start=(cg == 0), stop=(cg == G - 1))
            nc.scalar.activation(out=qt[:, dg, :], in_=pq, func=Tanh,
                                 bias=bq[:, dg:dg + 1], scale=1.0)

        for _rep in range(repeat):
            for b in range(NB):
                img = pimg32.tile([P, G, N], F32R, tag="img32")
                for cg in range(G):
                    nc.sync.dma_start(out=img[:, cg, :],
                                      in_=img_ap[b, ds(cg * P, P), :])

                keys = pkeys.tile([P, G, N], F32R, tag="keys")
                for dg in range(G):
                    for hf in range(2):
                        pk = ppmm.tile([P, 512], F32, tag="mm")
                        for cg in range(G):
                            mm(pk, wk[:, cg, ds(dg * P, P)],
                               img[:, cg, ds(hf * 512, 512)],
                               start=(cg == 0), stop=(cg == G - 1))
                        nc.scalar.activation(
                            out=keys[:, dg, ds(hf * 512, 512)], in_=pk,
                            func=Tanh, bias=bk[:, dg:dg + 1], scale=1.0)

                vals = pvals.tile([P, NB, D], F32R, tag="vals")
                for ch in range(NB):
                    pv = ppmm.tile([P, 512], F32, tag="mm")
                    for cg in range(G):
                        mm(pv, img[:, cg, ds(ch * P, P)], wv[:, cg, :],
                           start=(cg == 0), stop=(cg == G - 1))
                    nc.vector.tensor_add(out=pv, in0=pv, in1=bvb)
                    nc.scalar.activation(out=vals[:, ch, :], in_=pv, func=Tanh)

                wexp = psmall.tile([1, N], F32R, tag="wexp")
                s01 = psmall.tile([1, 2], F32, tag="s01")
                for hf in range(2):
                    psc = ppvec.tile([1, 512], F32, tag="vec")
                    for dg in range(G):
                        mm(psc, qt[:, dg, b:b + 1],
                           keys[:, dg, ds(hf * 512, 512)],
                           start=(dg == 0), stop=(dg == G - 1))
                    nc.scalar.activation(out=wexp[0:1, ds(hf * 512, 512)],
                                         in_=psc, func=Exp,
                                         accum_out=s01[0:1, hf:hf + 1])

                wt = psmall.tile([P, NB], F32R, tag="wt")
                for ch in range(NB):
                    pt = ppt.tile([P, 2], F32, tag="t")
                    mm(pt, wexp[0:1, ds(ch * P, P)], ones,
                       start=True, stop=True)
                    nc.vector.tensor_copy(out=wt[:, ch:ch + 1], in_=pt[:, 0:1])

                pc = ppvec.tile([1, D], F32, tag="vec")
                for ch in range(NB):
                    mm(pc, wt[:, ch:ch + 1], vals[:, ch, :],
                       start=(ch == 0), stop=(ch == NB - 1))

                stot = psmall.tile([1, 1], F32, tag="stot")
                nc.vector.tensor_add(out=stot, in0=s01[0:1, 0:1],
                                     in1=s01[0:1, 1:2])
                rtot = psmall.tile([1, 1], F32, tag="rtot")
                nc.vector.reciprocal(out=rtot, in_=stot)
                osb = psmall.tile([1, D], F32, tag="osb")
                nc.vector.tensor_scalar_mul(osb, pc, rtot)
                nc.sync.dma_start(out=out_ap[b:b + 1, :], in_=osb)

    nc.compile()
    return nc


def _build_f32r(repeat=1, img_internal=False):
    nc = bacc.Bacc("TRN2", target_bir_lowering=False, debug=False,
                   num_devices=NCORES)

    img_kind = "Internal" if img_internal else "ExternalInput"
    img_ap = nc.dram_tensor("img", [NB, D, N], F32R, kind=img_kind).ap()
    ht_ap = nc.dram_tensor("hT", [P, G, NB], F32R, kind="ExternalInput").ap()
    wq_ap = nc.dram_tensor("wqT", [P, G, D], F32R, kind="ExternalInput").ap()
    wk_ap = nc.dram_tensor("wkT", [P, G, D], F32R, kind="ExternalInput").ap()
    wv_ap = nc.dram_tensor("wvT", [P, G, D], F32R, kind="ExternalInput").ap()
    bq_ap = nc.dram_tensor("bqT", [P, G], F32, kind="ExternalInput").ap()
    bk_ap = nc.dram_tensor("bkT", [P, G], F32, kind="ExternalInput").ap()
    bv_ap = nc.dram_tensor("bvT", [P, G], F32, kind="ExternalInput").ap()
    # row of 128 ones; scaled by 1/sum(w~) it becomes the broadcast lhsT
    onesr_ap = nc.dram_tensor("onesr", [1, P], F32R, kind="ExternalInput").ap()
    out_ap = nc.dram_tensor("out", [NB, D], F32, kind="ExternalOutput").ap()

    mm = nc.tensor.matmul

    with tile.TileContext(nc) as tc, ExitStack() as ctx:
        consts = ctx.enter_context(tc.tile_pool(name="consts", bufs=1))
        pimg = ctx.enter_context(tc.tile_pool(name="pimg", bufs=2))
        pkeys = ctx.enter_context(tc.tile_pool(name="pkeys", bufs=2))
        pvals = ctx.enter_context(tc.tile_pool(name="pvals", bufs=2))
        pttr = ctx.enter_context(tc.tile_pool(name="pttr", bufs=3))
        psmall = ctx.enter_context(tc.tile_pool(name="psmall", bufs=4))
        ppmm = ctx.enter_context(tc.tile_pool(name="ppmm", bufs=3, space="PSUM"))
        ppvec = ctx.enter_context(tc.tile_pool(name="ppvec", bufs=2, space="PSUM"))
        ppbc = ctx.enter_context(tc.tile_pool(name="ppbc", bufs=2, space="PSUM"))

        # ---- constants ----
        wq = consts.tile([P, G, D], F32R, tag="wq")
        nc.sync.dma_start(out=wq, in_=wq_ap)
        wk = consts.tile([P, G, D], F32R, tag="wk")
        nc.sync.dma_start(out=wk, in_=wk_ap)
        wv = consts.tile([P, G, D], F32R, tag="wv")
        nc.sync.dma_start(out=wv, in_=wv_ap)
        bq = consts.tile([P, G], F32, tag="bq")
        nc.sync.dma_start(out=bq, in_=bq_ap)
        bk = consts.tile([P, G], F32, tag="bk")
        nc.sync.dma_start(out=bk, in_=bk_ap)
        bv = consts.tile([P, G], F32, tag="bv")
        nc.sync.dma_start(out=bv, in_=bv_ap)
        onesr = consts.tile([1, P], F32R, tag="onesr")
        nc.sync.dma_start(out=onesr, in_=onesr_ap)
        ht = consts.tile([P, G, NB], F32R, tag="ht")
        nc.sync.dma_start(out=ht, in_=ht_ap)

        # ---- queries for all local batches: qt[p, dg, b] ----
        qt = consts.tile([P, G, NB], F32R, tag="qt")
        for dg in range(G):
            pq = ppvec.tile([P, NB], F32, tag="vec")
            for cg in range(G):
                mm(pq, lhsT=wq[:, cg, ds(dg * P, P)], rhs=ht[:, cg, :],
                   start=(cg == 0), stop=(cg == G - 1))
            nc.scalar.activation(out=qt[:, dg, :], in_=pq, func=Tanh,
                                 bias=bq[:, dg:dg + 1], scale=1.0)

        # ---- per-batch pipeline ----
        for _rep in range(repeat):
            for b in range(NB):
                img = pimg.tile([P, G, N], F32R, tag="img")
                for cg in range(G):
                    nc.sync.dma_start(out=img[:, cg, :],
                                      in_=img_ap[b, ds(cg * P, P), :])

                # keysT / valsT [d, n] = tanh(W @ x + bias), fused on ScalarE
                keys = pkeys.tile([P, G, N], F32R, tag="keys")
                vals = pvals.tile([P, G, N], F32, tag="vals")
                for dg in range(G):
                    for hf in range(2):
                        pk = ppmm.tile([P, 512], F32, tag="mm")
                        for cg in range(G):
                            mm(pk, lhsT=wk[:, cg, ds(dg * P, P)],
                               rhs=img[:, cg, ds(hf * 512, 512)],
                               start=(cg == 0), stop=(cg == G - 1))
                        nc.scalar.activation(
                            out=keys[:, dg, ds(hf * 512, 512)], in_=pk,
                            func=Tanh, bias=bk[:, dg:dg + 1], scale=1.0)
                        pv = ppmm.tile([P, 512], F32, tag="mm")
                        for cg in range(G):
                            mm(pv, lhsT=wv[:, cg, ds(dg * P, P)],
                               rhs=img[:, cg, ds(hf * 512, 512)],
                               start=(cg == 0), stop=(cg == G - 1))
                        nc.scalar.activation(
                            out=vals[:, dg, ds(hf * 512, 512)], in_=pv,
                            func=Tanh, bias=bv[:, dg:dg + 1], scale=1.0)

                # scores[n] = q . keysT[:, n]; w~ = exp(scores), sum on the fly
                wexp = psmall.tile([1, N], F32R, tag="wexp")
                s01 = psmall.tile([1, 2], F32, tag="s01")
                for hf in range(2):
                    psc = ppvec.tile([1, 512], F32, tag="vec")
                    for dg in range(G):
                        mm(psc, lhsT=qt[:, dg, b:b + 1],
                           rhs=keys[:, dg, ds(hf * 512, 512)],
                           start=(dg == 0), stop=(dg == G - 1))
                    nc.scalar.activation(out=wexp[0:1, ds(hf * 512, 512)],
                                         in_=psc, func=Exp,
                                         accum_out=s01[0:1, hf:hf + 1])

                # 1/sum(w~) as a 128-wide f32r row for the broadcast matmul
                stot = psmall.tile([1, 1], F32, tag="stot")
                nc.vector.tensor_add(out=stot, in0=s01[0:1, 0:1],
                                     in1=s01[0:1, 1:2])
                rtot = psmall.tile([1, 1], F32, tag="rtot")
                nc.vector.reciprocal(out=rtot, in_=stot)
                rrow = psmall.tile([1, P], F32R, tag="rrow")
                nc.vector.tensor_scalar_mul(rrow, onesr, rtot)

                # context[d] = sum_n w[n] valsT[d, n] via broadcast + DVE
                # reduce: pb[m, n] = w~[n]/sum  for all partitions m
                # tensor_tensor_reduce needs a custom DVE table the runtime
                # can't load here; DVE multiply + ScalarE Copy-with-accum is
                # equivalent and uses only HW-proven constructs.
                ctxh = psmall.tile([P, G, 2], F32, tag="ctxh")
                for hf in range(2):
                    pb = ppbc.tile([P, 512], F32, tag="bc")
                    mm(pb, lhsT=rrow, rhs=wexp[0:1, ds(hf * 512, 512)],
                       start=True, stop=True)
                    for dg in range(G):
                        tout = pttr.tile([P, 512], F32, tag="ttr")
                        nc.vector.tensor_mul(tout,
                                             vals[:, dg, ds(hf * 512, 512)],
                                             pb)
                        nc.scalar.activation(
                            out=tout, in_=tout, func=Copy,
                            accum_out=ctxh[:, dg, hf:hf + 1])

                ctxs = psmall.tile([P, G], F32, tag="ctxs")
                nc.vector.tensor_add(out=ctxs, in0=ctxh[:, :, 0],
                                     in1=ctxh[:, :, 1])
                out_view = out_ap[b:b + 1, :].rearrange(
                    "a (g p) -> p (a g)", p=P)
                nc.sync.dma_start(out=out_view, in_=ctxs)

    nc.compile()
    return nc


def _build_bf16(repeat=1, img_internal=False):
    nc = bacc.Bacc("TRN2", target_bir_lowering=False, debug=False,
                   num_devices=NCORES)

    img_kind = "Internal" if img_internal else "ExternalInput"
    img_ap = nc.dram_tensor("img", [NB, D, N], F32, kind=img_kind).ap()
    ht_ap = nc.dram_tensor("hT", [P, G, NB], F32, kind="ExternalInput").ap()
    wq_ap = nc.dram_tensor("wqT", [P, G, D], BF, kind="ExternalInput").ap()
    wk_ap = nc.dram_tensor("wkT", [P, G, D], BF, kind="ExternalInput").ap()
    wv_ap = nc.dram_tensor("wvT", [P, G, D], BF, kind="ExternalInput").ap()
    bq_ap = nc.dram_tensor("bqT", [P, G], F32, kind="ExternalInput").ap()
    bk_ap = nc.dram_tensor("bkT", [P, G], F32, kind="ExternalInput").ap()
    bvb_ap = nc.dram_tensor("bvb", [P, D], F32, kind="ExternalInput").ap()
    out_ap = nc.dram_tensor("out", [NB, D], F32, kind="ExternalOutput").ap()

    def mm(out, lhsT, rhs, start, stop):
        nc.tensor.matmul(out, lhsT=lhsT, rhs=rhs, start=start, stop=stop)

    with tile.TileContext(nc) as tc, ExitStack() as ctx:
        consts = ctx.enter_context(tc.tile_pool(name="consts", bufs=1))
        pimg32 = ctx.enter_context(tc.tile_pool(name="pimg32", bufs=2))
        pimg16 = ctx.enter_context(tc.tile_pool(name="pimg16", bufs=2))
        pkeys = ctx.enter_context(tc.tile_pool(name="pkeys", bufs=2))
        pvals = ctx.enter_context(tc.tile_pool(name="pvals", bufs=2))
        psmall = ctx.enter_context(tc.tile_pool(name="psmall", bufs=4))
        ppmm = ctx.enter_context(tc.tile_pool(name="ppmm", bufs=3, space="PSUM"))
        ppvec = ctx.enter_context(tc.tile_pool(name="ppvec", bufs=3, space="PSUM"))
        ppt = ctx.enter_context(tc.tile_pool(name="ppt", bufs=2, space="PSUM"))

        wq = consts.tile([P, G, D], BF, tag="wq")
        nc.sync.dma_start(out=wq, in_=wq_ap)
        wk = consts.tile([P, G, D], BF, tag="wk")
        nc.sync.dma_start(out=wk, in_=wk_ap)
        wv = consts.tile([P, G, D], BF, tag="wv")
        nc.sync.dma_start(out=wv, in_=wv_ap)
        bq = consts.tile([P, G], F32, tag="bq")
        nc.sync.dma_start(out=bq, in_=bq_ap)
        bk = consts.tile([P, G], F32, tag="bk")
        nc.sync.dma_start(out=bk, in_=bk_ap)
        bvb = consts.tile([P, D], F32, tag="bvb")
        nc.sync.dma_start(out=bvb, in_=bvb_ap)
        ht32 = consts.tile([P, G, NB], F32, tag="ht32")
        nc.sync.dma_start(out=ht32, in_=ht_ap)
        ones = consts.tile([1, 1], BF, tag="ones")
        nc.vector.memset(ones, 1.0)
        ht = consts.tile([P, G, NB], BF, tag="ht")
        nc.vector.tensor_copy(out=ht, in_=ht32)

        qt = consts.tile([P, G, NB], BF, tag="qt")
        for dg in range(G):
            pq = ppt.tile([P, NB], F32, tag="t")
            for cg in range(G):
                mm(pq, wq[:, cg, ds(dg * P, P)], ht[:, cg, :],
                   start=(cg == 0), stop=(cg == G - 1))
            nc.scalar.activation(out=qt[:, dg, :], in_=pq, func=Tanh,
                                 bias=bq[:, dg:dg + 1], scale=1.0)

        for _rep in range(repeat):
            for b in range(NB):
                img32 = pimg32.tile([P, G, N], F32, tag="img32")
                for cg in range(G):
                    nc.sync.dma_start(out=img32[:, cg, :],
                                      in_=img_ap[b, ds(cg * P, P), :])
                img16 = pimg16.tile([P, G, N], BF, tag="img16")
                for cg in range(G):
                    nc.vector.tensor_copy(out=img16[:, cg, :],
                                          in_=img32[:, cg, :])

                keys = pkeys.tile([P, G, N], BF, tag="keys")
                for dg in range(G):
                    for hf in range(2):
                        pk = ppmm.tile([P, 512], F32, tag="mm")
                        for cg in range(G):
                            mm(pk, wk[:, cg, ds(dg * P, P)],
                               img16[:, cg, ds(hf * 512, 512)],
                               start=(cg == 0), stop=(cg == G - 1))
                        nc.scalar.activation(
                            out=keys[:, dg, ds(hf * 512, 512)], in_=pk,
                            func=Tanh, bias=bk[:, dg:dg + 1], scale=1.0)

                vals = pvals.tile([P, NB, D], BF, tag="vals")
                for ch in range(NB):
                    pv = ppmm.tile([P, 512], F32, tag="mm")
                    for cg in range(G):
                        mm(pv, img16[:, cg, ds(ch * P, P)], wv[:, cg, :],
                           start=(cg == 0), stop=(cg == G - 1))
                    nc.vector.tensor_add(out=pv, in0=pv, in1=bvb)
                    nc.scalar.activation(out=vals[:, ch, :], in_=pv, func=Tanh)

                wexp = psmall.tile([1, N], BF, tag="wexp")
                s01 = psmall.tile([1, 2], F32, tag="s01")
                for hf in range(2):
                    psc = ppvec.tile([1, 512], F32, tag="vec")
                    for dg in range(G):
                        mm(psc, qt[:, dg, b:b + 1],
                           keys[:, dg, ds(hf * 512, 512)],
                           start=(dg == 0), stop=(dg == G - 1))
                    nc.scalar.activation(out=wexp[0:1, ds(hf * 512, 512)],
                                         in_=psc, func=Exp,
                                         accum_out=s01[0:1, hf:hf + 1])

                wt = psmall.tile([P, NB], BF, tag="wt")
                for ch in range(NB):
                    pt = ppt.tile([P, 1], F32, tag="t")
                    mm(pt, wexp[0:1, ds(ch * P, P)], ones,
                       start=True, stop=True)
                    nc.vector.tensor_copy(out=wt[:, ch:ch + 1], in_=pt[:, 0:1])

                pc = ppvec.tile([1, D], F32, tag="vec")
                for ch in range(NB):
                    mm(pc, wt[:, ch:ch + 1], vals[:, ch, :],
                       start=(ch == 0), stop=(ch == NB - 1))

                stot = psmall.tile([1, 1], F32, tag="stot")
                nc.vector.tensor_add(out=stot, in0=s01[0:1, 0:1],
                                     in1=s01[0:1, 1:2])
                rtot = psmall.tile([1, 1], F32, tag="rtot")
                nc.vector.reciprocal(out=rtot, in_=stot)
                osb = psmall.tile([1, D], F32, tag="osb")
                nc.vector.tensor_scalar_mul(osb, pc, rtot)
                nc.sync.dma_start(out=out_ap[b:b + 1, :], in_=osb)

    nc.compile()
    return nc


def _get_nc(mode=MODE, repeat=1, img_internal=False):
    key = (mode, repeat, img_internal)
    if key not in _CACHED:
        if mode == "bf16":
            _CACHED[key] = _build_bf16(repeat, img_internal)
        elif mode == "f32rv2":
            _CACHED[key] = _build_f32r(repeat, img_internal)
        else:
            _CACHED[key] = _build_f32r_v1(repeat, img_internal)
    return _CACHED[key]


def _weight_layout(W, mode):
    # [512, 512] W[d, c] -> [128, 4, 512] with w[p, g, d] = W[d, g*128+p]
    WT = np.ascontiguousarray(np.asarray(W, dtype=np.float32).T)  # [c, d]
    t = np.ascontiguousarray(WT.reshape(G, P, D).transpose(1, 0, 2))
    return t.astype(ml_dtypes.bfloat16) if mode == "bf16" else t


def _bias_layout(b):
    # [512] -> [128, 4] with out[p, g] = b[g*128 + p]
    return np.ascontiguousarray(
        np.asarray(b, dtype=np.float32).reshape(G, P).T)


def make_in_maps(channel_img, last_hidden_lstm, Wq, bq, Wk, bk, Wv, bv,
                 mode=MODE):
    channel_img = np.asarray(channel_img, dtype=np.float32)
    last_hidden_lstm = np.asarray(last_hidden_lstm, dtype=np.float32)
    B, C, H, W = channel_img.shape
    assert (B, C, H * W) == (NCORES * NB, D, N)
    img_full = channel_img.reshape(B, C, H * W)

    wqT = _weight_layout(Wq, mode)
    wkT = _weight_layout(Wk, mode)
    wvT = _weight_layout(Wv, mode)
    bqT = _bias_layout(bq)
    bkT = _bias_layout(bk)

    in_maps = []
    for i in range(NCORES):
        h = last_hidden_lstm[i * NB:(i + 1) * NB]        # [NB, 512]
        ht = np.ascontiguousarray(h.T.reshape(G, P, NB).transpose(1, 0, 2))
        m = {
            "img": np.ascontiguousarray(img_full[i * NB:(i + 1) * NB]),
            "hT": ht,
            "wqT": wqT, "wkT": wkT, "wvT": wvT,
            "bqT": bqT, "bkT": bkT,
        }
        if mode == "bf16":
            m["bvb"] = np.ascontiguousarray(
                np.broadcast_to(np.asarray(bv, dtype=np.float32), (P, D)))
        elif mode == "f32rv2":
            m["bvT"] = _bias_layout(bv)
            m["onesr"] = np.ones((1, P), np.float32)
        else:
            m["bvb"] = np.ascontiguousarray(
                np.broadcast_to(np.asarray(bv, dtype=np.float32), (P, D)))
            m["onec"] = np.array([[1.0, 0.0]], np.float32)
        in_maps.append(m)
    return in_maps


def run(in_maps, mode=MODE, repeat=1, **kwargs):
    nc = _get_nc(mode, repeat)
    res = run_bass_kernel_spmd(nc, in_maps, core_ids=list(range(NCORES)),
                               **kwargs)
    out = np.concatenate([res.results[i]["out"] for i in range(NCORES)], axis=0)
    return np.ascontiguousarray(out.astype(np.float32)), res


def kernel(channel_img, last_hidden_lstm, Wq, bq, Wk, bk, Wv, bv):
    in_maps = make_in_maps(channel_img, last_hidden_lstm,
                           Wq, bq, Wk, bk, Wv, bv, mode=MODE)
    out, _ = run(in_maps, mode=MODE)
    return out



# revision 13
# speedup vs baseline: 1.5632x; 1.5632x over previous
"""Cross-attention kernel for Trainium2, SPMD over 8 NeuronCores.

Reference computation (per batch b):
    x       = channel_img[b].reshape(C, N)          # [512, 1024], N = 32*32
    query   = tanh(Wq @ h[b] + bq)                  # [512]
    keysT   = tanh(Wk @ x + bk[:, None])            # [512, 1024]   (d, n)
    valsT   = tanh(Wv @ x + bv[:, None])            # [512, 1024]   (d, n)
    scores  = query @ keysT                         # [1024]
    w       = softmax(scores)
    out[b]  = valsT @ w                             # [512]

Sharding: data-parallel over batch, 8 batches per core, weights replicated.

f32r design (default): all matmul operands are float32r (fp32 storage,
reduced-precision PE mode, 1 cyc/row at moving dim >= 256 — same speed as
bf16 but ~16x more accurate on HW). Per batch:
  - keys/values projections: 64 matmuls N=512, bias+tanh fused on ScalarE
  - scores: M=1 matvec (8 matmuls N=512), exp + running sum fused on ScalarE
  - context: normalized w broadcast to all partitions via one K=1 matmul per
    half (lhsT = row of 1/sum(w)), then VectorE tensor_tensor_reduce does
    the weighted reduction valsT . w along the free dim — no PE transposes,
    no context matvec on PE.

bf16 fallback design keeps values in [n, d] orientation with a PE context
matvec and PE-based w transposes.
"""

import numpy as np
import ml_dtypes
from contextlib import ExitStack

import concourse.bass as bass
import concourse.tile as tile
from concourse import bacc, mybir
from concourse.bass import ds
from concourse.bass_utils import run_bass_kernel_spmd

P = 128          # SBUF partitions
G = 4            # 512 = G * P groups along the hidden dim
D = 512          # hidden size
N = 1024         # spatial positions (32*32)
NB = 8           # batches per core
NCORES = 8
BF = mybir.dt.bfloat16
F32 = mybir.dt.float32
F32R = mybir.dt.float32r
Tanh = mybir.ActivationFunctionType.Tanh
Exp = mybir.ActivationFunctionType.Exp
Copy = mybir.ActivationFunctionType.Copy
Mult = mybir.AluOpType.mult
Add = mybir.AluOpType.add

MODE = "dr8"     # default mode used by kernel()

E4 = mybir.dt.float8e4
DRow = mybir.MatmulPerfMode.DoubleRow
XS = 8.0         # img fp8 scale
WS = 128.0       # weight fp8 scale

_CACHED = {}


def _build_dr8(repeat=1, img_internal=False):
    """fp8 DoubleRow design, software-pipelined emission.

    Per batch:
      keysT[d, n] = tanh((Wk8 @ x8) / (XS*WS) + bk): 12 DR matmuls per dg
        group (3-term hi/lo product split) into a 2-bank psum, one
        [128, 1024] Act per group fusing scale+bias.
      vals[n, d]: 6 DR matmuls per [128, 512] tile; bias added by DVE
        (bvb broadcast tile) in psum; Act tanh fuses the 1/(XS*WS) scale.
      scores[n] = keys . q: 32 out-free-1 matmuls into aux psum cols 0-7;
        Act Exp -> unnormalized bf16 weights + per-partition sums.
      sum/recip: 2 tiny matmuls (partition reduce, broadcast) + DVE.
      ctx[d] = vals^T w~: 32 out-free-1 matmuls into aux cols 12-15; DVE
        applies 1/sum and writes the output row.
    PE tiny matmuls are woven into the DR streams of the neighbouring
    stages so the tensor engine never idles; Act/DVE run in the shadow.
    """
    nc = bacc.Bacc("TRN2", target_bir_lowering=False, debug=False,
                   num_devices=NCORES)

    img_kind = "Internal" if img_internal else "ExternalInput"
    # [b, hi/lo, c-part, cg, n]
    img_ap = nc.dram_tensor("img8", [NB, 2, P, G, N], E4, kind=img_kind).ap()
    wkh_ap = nc.dram_tensor("wkh", [P, G, D], E4, kind="ExternalInput").ap()
    wkl_ap = nc.dram_tensor("wkl", [P, G, D], E4, kind="ExternalInput").ap()
    wvh_ap = nc.dram_tensor("wvh", [P, G, D], E4, kind="ExternalInput").ap()
    wvl_ap = nc.dram_tensor("wvl", [P, G, D], E4, kind="ExternalInput").ap()
    bvb_ap = nc.dram_tensor("bvb", [P, D], F32, kind="ExternalInput").ap()
    wq_ap = nc.dram_tensor("wqT", [P, G, D], F32R, kind="ExternalInput").ap()
    ht_ap = nc.dram_tensor("hT", [P, G, NB], F32R, kind="ExternalInput").ap()
    bq_ap = nc.dram_tensor("bqT", [P, G], F32, kind="ExternalInput").ap()
    bk_ap = nc.dram_tensor("bkT", [P, G], F32, kind="ExternalInput").ap()
    onc_ap = nc.dram_tensor("onc", [P, 1], F32, kind="ExternalInput").ap()
    onr_ap = nc.dram_tensor("onr", [1, P], F32, kind="ExternalInput").ap()
    out_ap = nc.dram_tensor("out", [NB, P, G], F32, kind="ExternalOutput").ap()

    mm = nc.tensor.matmul
    inv = 1.0 / (XS * WS)

    with tile.TileContext(nc) as tc, ExitStack() as ctx:
        consts = ctx.enter_context(tc.tile_pool(name="consts", bufs=1))
        pimg = ctx.enter_context(tc.tile_pool(name="pimg", bufs=3))
        pkeys = ctx.enter_context(tc.tile_pool(name="pkeys", bufs=2))
        pvals = ctx.enter_context(tc.tile_pool(name="pvals", bufs=2))
        psmall = ctx.enter_context(tc.tile_pool(name="psmall", bufs=4))
        ppk = ctx.enter_context(tc.tile_pool(name="ppk", bufs=2, space="PSUM"))
        ppv = ctx.enter_context(tc.tile_pool(name="ppv", bufs=3, space="PSUM"))
        ppa = ctx.enter_context(tc.tile_pool(name="ppa", bufs=1, space="PSUM"))

        # DMA order: batch-0 image + K weights first so keys(0) starts ASAP.
        xh0 = pimg.tile([P, G, N], E4, tag="xh")
        nc.sync.dma_start(out=xh0[:, 0:2, :], in_=img_ap[0, 0, :, 0:2, :])
        wkh = consts.tile([P, G, D], E4, tag="wkh")
        nc.sync.dma_start(out=wkh, in_=wkh_ap)
        nc.sync.dma_start(out=xh0[:, 2:4, :], in_=img_ap[0, 0, :, 2:4, :])
        wkl = consts.tile([P, G, D], E4, tag="wkl")
        nc.sync.dma_start(out=wkl, in_=wkl_ap)
        xl0 = pimg.tile([P, G, N], E4, tag="xl")
        nc.sync.dma_start(out=xl0, in_=img_ap[0, 1])
        bk = consts.tile([P, G], F32, tag="bk")
        nc.sync.dma_start(out=bk, in_=bk_ap)
        wvh = consts.tile([P, G, D], E4, tag="wvh")
        nc.sync.dma_start(out=wvh, in_=wvh_ap)
        wvl = consts.tile([P, G, D], E4, tag="wvl")
        nc.sync.dma_start(out=wvl, in_=wvl_ap)
        bvb = consts.tile([P, D], F32, tag="bvb")
        nc.sync.dma_start(out=bvb, in_=bvb_ap)
        wq = consts.tile([P, G, D], F32R, tag="wq")
        nc.sync.dma_start(out=wq, in_=wq_ap)
        ht = consts.tile([P, G, NB], F32R, tag="ht")
        nc.sync.dma_start(out=ht, in_=ht_ap)
        bq = consts.tile([P, G], F32, tag="bq")
        nc.sync.dma_start(out=bq, in_=bq_ap)
        onc = consts.tile([P, 1], F32, tag="onc")
        nc.sync.dma_start(out=onc, in_=onc_ap)
        onr = consts.tile([1, P], F32, tag="onr")
        nc.sync.dma_start(out=onr, in_=onr_ap)

        qt = consts.tile([P, G, NB], BF, tag="qt")

        st = {}          # per-batch live tiles

        def dma_img(b):
            xh = pimg.tile([P, G, N], E4, tag="xh")
            nc.sync.dma_start(out=xh, in_=img_ap[b, 0])
            xl = pimg.tile([P, G, N], E4, tag="xl")
            nc.sync.dma_start(out=xl, in_=img_ap[b, 1])
            st[b] = {"xh": xh, "xl": xl}

        def gen_query():
            qa = ppa.tile([P, 48], F32, tag="a")
            for dg in range(G):
                sl = ds(16 + dg * NB, NB)
                for cg in range(G):
                    mm(qa[:, sl], lhsT=wq[:, cg, ds(dg * P, P)],
                       rhs=ht[:, cg, :],
                       start=(cg == 0), stop=(cg == G - 1))
                    yield
                nc.scalar.activation(out=qt[:, dg, :], in_=qa[:, sl],
                                     func=Tanh, bias=bq[:, dg:dg + 1],
                                     scale=1.0)

        def gen_keys(b):
            xh, xl = st[b]["xh"], st[b]["xl"]
            keys = pkeys.tile([P, G, N], BF, tag="keys")
            st[b]["keys"] = keys
            for dg in range(G):
                dgs = ds(dg * P, P)
                pk = ppk.tile([P, 2, 512], F32, tag="k")
                # xh-dependent terms first (batch 0: xl lands later)
                for hf in range(2):
                    hfs = ds(hf * 512, 512)
                    mm(pk[:, hf, :], lhsT=wkh[:, 0:2, dgs],
                       rhs=xh[:, 0:2, hfs],
                       start=True, stop=False, perf_mode=DRow)
                    yield
                    mm(pk[:, hf, :], lhsT=wkh[:, 2:4, dgs],
                       rhs=xh[:, 2:4, hfs],
                       start=False, stop=False, perf_mode=DRow)
                    yield
                    mm(pk[:, hf, :], lhsT=wkl[:, 0:2, dgs],
                       rhs=xh[:, 0:2, hfs],
                       start=False, stop=False, perf_mode=DRow)
                    yield
                    mm(pk[:, hf, :], lhsT=wkl[:, 2:4, dgs],
                       rhs=xh[:, 2:4, hfs],
                       start=False, stop=False, perf_mode=DRow)
                    yield
                for hf in range(2):
                    hfs = ds(hf * 512, 512)
                    mm(pk[:, hf, :], lhsT=wkh[:, 0:2, dgs],
                       rhs=xl[:, 0:2, hfs],
                       start=False, stop=False, perf_mode=DRow)
                    yield
                    mm(pk[:, hf, :], lhsT=wkh[:, 2:4, dgs],
                       rhs=xl[:, 2:4, hfs],
                       start=False, stop=True, perf_mode=DRow)
                    yield
                nc.scalar.activation(
                    out=keys[:, dg, :], in_=pk, func=Tanh,
                    bias=bk[:, dg:dg + 1], scale=inv)

        def gen_vals(b):
            xh, xl = st[b]["xh"], st[b]["xl"]
            vals = pvals.tile([P, NB, D], BF, tag="vals")
            st[b]["vals"] = vals
            for nb in range(NB):
                nbs = ds(nb * P, P)
                pv = ppv.tile([P, D], F32, tag="v")
                mm(pv, lhsT=xh[:, 0:2, nbs], rhs=wvh[:, 0:2, :],
                   start=True, stop=False, perf_mode=DRow)
                yield
                mm(pv, lhsT=xh[:, 2:4, nbs], rhs=wvh[:, 2:4, :],
                   start=False, stop=False, perf_mode=DRow)
                yield
                mm(pv, lhsT=xl[:, 0:2, nbs], rhs=wvh[:, 0:2, :],
                   start=False, stop=False, perf_mode=DRow)
                yield
                mm(pv, lhsT=xl[:, 2:4, nbs], rhs=wvh[:, 2:4, :],
                   start=False, stop=False, perf_mode=DRow)
                yield
                mm(pv, lhsT=xh[:, 0:2, nbs], rhs=wvl[:, 0:2, :],
                   start=False, stop=False, perf_mode=DRow)
                yield
                mm(pv, lhsT=xh[:, 2:4, nbs], rhs=wvl[:, 2:4, :],
                   start=False, stop=True, perf_mode=DRow)
                yield
                nc.vector.tensor_add(out=pv, in0=pv, in1=bvb)
                nc.scalar.activation(out=vals[:, nb, :], in_=pv,
                                     func=Tanh, scale=inv)

        def gen_scores(b):
            keys = st[b]["keys"]
            pa = ppa.tile([P, 48], F32, tag="a")
            st[b]["pa"] = pa
            for nb in range(NB):
                for dg in range(G):
                    mm(pa[:, nb:nb + 1],
                       lhsT=keys[:, dg, ds(nb * P, P)],
                       rhs=qt[:, dg, b:b + 1],
                       start=(dg == 0), stop=(dg == G - 1))
                    yield

        def emit_exp(b):
            pa = st[b]["pa"]
            wbf = psmall.tile([P, NB], BF, tag="wbf")
            s1 = psmall.tile([P, 1], F32, tag="s1")
            nc.scalar.activation(out=wbf, in_=pa[:, 0:NB], func=Exp,
                                 accum_out=s1)
            st[b]["wbf"] = wbf
            st[b]["s1"] = s1

        def gen_aux(b):
            pa, s1 = st[b]["pa"], st[b]["s1"]
            mm(pa[0:1, 8:9], lhsT=s1, rhs=onc, start=True, stop=True)
            yield
            rtot = psmall.tile([1, 1], F32, tag="rtot")
            nc.vector.reciprocal(out=rtot, in_=pa[0:1, 8:9])
            st[b]["rtot"] = rtot

        def gen_bcast(b):
            pa, rtot = st[b]["pa"], st[b]["rtot"]
            mm(pa[:, 9:10], lhsT=onr, rhs=rtot, start=True, stop=True)
            yield
            rbs = psmall.tile([P, 1], F32, tag="rbs")
            nc.vector.tensor_copy(out=rbs, in_=pa[:, 9:10])
            st[b]["rbs"] = rbs

        def gen_ctx(b):
            pa, vals, wbf = st[b]["pa"], st[b]["vals"], st[b]["wbf"]
            for dg in range(G):
                for nb in range(NB):
                    mm(pa[:, 12 + dg:13 + dg],
                       lhsT=vals[:, nb, ds(dg * P, P)],
                       rhs=wbf[:, nb:nb + 1],
                       start=(nb == 0), stop=(nb == NB - 1))
                    yield

        def emit_out(b):
            pa, rbs = st[b]["pa"], st[b]["rbs"]
            osb = psmall.tile([P, G], F32, tag="osb")
            nc.vector.tensor_scalar_mul(osb, pa[:, 12:16], rbs)
            nc.sync.dma_start(out=out_ap[b], in_=osb)
            del st[b]

        def drive(gen, n=None):
            """Emit up to n PE instructions from gen; True if exhausted."""
            try:
                if n is None:
                    while True:
                        next(gen)
                else:
                    for _ in range(n):
                        next(gen)
            except StopIteration:
                return True
            return False

        def weave(main, *others):
            """Drain main, interleaving one instr of each other per step."""
            done_o = [False] * len(others)
            while not drive(main, 1):
                for i, o in enumerate(others):
                    if not done_o[i]:
                        done_o[i] = drive(o, 1)
            for o in others:
                drive(o)

        # ---- emission schedule ----
        assert repeat == 1 or img_internal
        st[0] = {"xh": xh0, "xl": xl0}
        for _rep in range(repeat):
            for b in range(NB):
                K = gen_keys(b)
                if b == 0:
                    # batch 0 prologue: keys, then query (first rep only)
                    drive(K)
                    if _rep == 0:
                        drive(gen_query())
                else:
                    # weave prev batch epilogue into keys(b) DR stream
                    drive(K, 10)
                    drive(gen_aux(b - 1))       # stot mm + DVE recip
                    drive(K, 6)
                    drive(gen_bcast(b - 1))     # bcast mm + DVE copy
                    weave(K, gen_ctx(b - 1))
                    emit_out(b - 1)
                if b + 1 < NB:
                    dma_img(b + 1)
                elif img_internal and _rep + 1 < repeat:
                    dma_img(0)
                V = gen_vals(b)
                drive(V, 16)
                weave(V, gen_scores(b))
                emit_exp(b)
            # tail: batch NB-1 epilogue; ctx overlaps the recip chain
            drive(gen_aux(NB - 1))
            drive(gen_ctx(NB - 1))
            drive(gen_bcast(NB - 1))
            emit_out(NB - 1)

    nc.compile()
    return nc


def _build_f32r_v1(repeat=1, img_internal=False,
                   bmm=4, bvec=2, bt=2, bimg=2, bkv=2, bsm=4):
    """HW-validated f32r design: values in [n, d] orientation, PE context
    matvec, w~ transposed via K=1/N=2 matmuls. ~141 us/rep steady-state."""
    nc = bacc.Bacc("TRN2", target_bir_lowering=False, debug=False,
                   num_devices=NCORES)

    img_kind = "Internal" if img_internal else "ExternalInput"
    img_ap = nc.dram_tensor("img", [NB, D, N], F32R, kind=img_kind).ap()
    ht_ap = nc.dram_tensor("hT", [P, G, NB], F32R, kind="ExternalInput").ap()
    wq_ap = nc.dram_tensor("wqT", [P, G, D], F32R, kind="ExternalInput").ap()
    wk_ap = nc.dram_tensor("wkT", [P, G, D], F32R, kind="ExternalInput").ap()
    wv_ap = nc.dram_tensor("wvT", [P, G, D], F32R, kind="ExternalInput").ap()
    bq_ap = nc.dram_tensor("bqT", [P, G], F32, kind="ExternalInput").ap()
    bk_ap = nc.dram_tensor("bkT", [P, G], F32, kind="ExternalInput").ap()
    bvb_ap = nc.dram_tensor("bvb", [P, D], F32, kind="ExternalInput").ap()
    one_ap = nc.dram_tensor("onec", [1, 2], F32R, kind="ExternalInput").ap()
    out_ap = nc.dram_tensor("out", [NB, D], F32, kind="ExternalOutput").ap()

    def mm(out, lhsT, rhs, start, stop):
        nc.tensor.matmul(out, lhsT=lhsT, rhs=rhs, start=start, stop=stop)

    with tile.TileContext(nc) as tc, ExitStack() as ctx:
        consts = ctx.enter_context(tc.tile_pool(name="consts", bufs=1))
        pimg32 = ctx.enter_context(tc.tile_pool(name="pimg32", bufs=bimg))
        pkeys = ctx.enter_context(tc.tile_pool(name="pkeys", bufs=bkv))
        pvals = ctx.enter_context(tc.tile_pool(name="pvals", bufs=bkv))
        psmall = ctx.enter_context(tc.tile_pool(name="psmall", bufs=bsm))
        ppmm = ctx.enter_context(tc.tile_pool(name="ppmm", bufs=bmm, space="PSUM"))
        ppvec = ctx.enter_context(tc.tile_pool(name="ppvec", bufs=bvec, space="PSUM"))
        ppt = ctx.enter_context(tc.tile_pool(name="ppt", bufs=bt, space="PSUM"))

        wq = consts.tile([P, G, D], F32R, tag="wq")
        nc.sync.dma_start(out=wq, in_=wq_ap)
        wk = consts.tile([P, G, D], F32R, tag="wk")
        nc.sync.dma_start(out=wk, in_=wk_ap)
        wv = consts.tile([P, G, D], F32R, tag="wv")
        nc.sync.dma_start(out=wv, in_=wv_ap)
        bq = consts.tile([P, G], F32, tag="bq")
        nc.sync.dma_start(out=bq, in_=bq_ap)
        bk = consts.tile([P, G], F32, tag="bk")
        nc.sync.dma_start(out=bk, in_=bk_ap)
        bvb = consts.tile([P, D], F32, tag="bvb")
        nc.sync.dma_start(out=bvb, in_=bvb_ap)
        ones = consts.tile([1, 2], F32R, tag="ones")
        nc.sync.dma_start(out=ones, in_=one_ap)
        ht = consts.tile([P, G, NB], F32R, tag="ht")
        nc.sync.dma_start(out=ht, in_=ht_ap)

        qt = consts.tile([P, G, NB], F32R, tag="qt")
        for dg in range(G):
            pq = ppt.tile([P, NB], F32, tag="t")
            for cg in range(G):
                mm(pq, wq[:, cg, ds(dg * P, P)], ht[:, cg, :],
                   start=(cg == 0), stop=(cg == G - 1))
            nc.scalar.activation(out=qt[:, dg, :], in_=pq, func=Tanh,
                                 bias=bq[:, dg:dg + 1], scale=1.0)

        for _rep in range(repeat):
            for b in range(NB):
                img = pimg32.tile([P, G, N], F32R, tag="img32")
                for cg in range(G):
                    nc.sync.dma_start(out=img[:, cg, :],
                                      in_=img_ap[b, ds(cg * P, P), :])

                keys = pkeys.tile([P, G, N], F32R, tag="keys")
                for dg in range(G):
                    for hf in range(2):
                        pk = ppmm.tile([P, 512], F32, tag="mm")
                        for cg in range(G):
                            mm(pk, wk[:, cg, ds(dg * P, P)],
                               img[:, cg, ds(hf * 512, 512)],
                               start=(cg == 0), stop=(cg == G - 1))
                        nc.scalar.activation(
                            out=keys[:, dg, ds(hf * 512, 512)], in_=pk,
                            func=Tanh, bias=bk[:, dg:dg + 1], scale=1.0)

                vals = pvals.tile([P, NB, D], F32R, tag="vals")
                for ch in range(NB):
                    pv = ppmm.tile([P, 512], F32, tag="mm")
                    for cg in range(G):
                        mm(pv, img[:, cg, ds(ch * P, P)], wv[:, cg, :],
                           start=(cg == 0), stop=(cg == G - 1))
                    nc.vector.tensor_add(out=pv, in0=pv, in1=bvb)
                    nc.scalar.activation(out=vals[:, ch, :], in_=pv, func=Tanh)

                wexp = psmall.tile([1, N], F32R, tag="wexp")
                s01 = psmall.tile([1, 2], F32, tag="s01")
                for hf in range(2):
                    psc = ppvec.tile([1, 512], F32, tag="vec")
                    for dg in range(G):
                        mm(psc, qt[:, dg, b:b + 1],
                           keys[:, dg, ds(hf * 512, 512)],
                           start=(dg == 0), stop=(dg == G - 1))
                    nc.scalar.activation(out=wexp[0:1, ds(hf * 512, 512)],
                                         in_=psc, func=Exp,
                                         accum_out=s01[0:1, hf:hf + 1])

                wt = psmall.tile([P, NB], F32R, tag="wt")
                for ch in range(NB):
                    pt = ppt.tile([P, 2], F32, tag="t")
                    mm(pt, wexp[0:1, ds(ch * P, P)], ones,
                       start=True, stop=True)
                    nc.vector.tensor_copy(out=wt[:, ch:ch + 1], in_=pt[:, 0:1])

                pc = ppvec.tile([1, D], F32, tag="vec")
                for ch in range(NB):
                    mm(pc, wt[:, ch:ch + 1], vals[:, ch, :],
                       start=(ch == 0), stop=(ch == NB - 1))

                stot = psmall.tile([1, 1], F32, tag="stot")
                nc.vector.tensor_add(out=stot, in0=s01[0:1, 0:1],
                                     in1=s01[0:1, 1:2])
                rtot = psmall.tile([1, 1], F32, tag="rtot")
                nc.vector.reciprocal(out=rtot, in_=stot)
                osb = psmall.tile([1, D], F32, tag="osb")
                nc.vector.tensor_scalar_mul(osb, pc, rtot)
                nc.sync.dma_start(out=out_ap[b:b + 1, :], in_=osb)

    nc.compile()
    return nc


def _build_f32r(repeat=1, img_internal=False):
    nc = bacc.Bacc("TRN2", target_bir_lowering=False, debug=False,
                   num_devices=NCORES)

    img_kind = "Internal" if img_internal else "ExternalInput"
    img_ap = nc.dram_tensor("img", [NB, D, N], F32R, kind=img_kind).ap()
    ht_ap = nc.dram_tensor("hT", [P, G, NB], F32R, kind="ExternalInput").ap()
    wq_ap = nc.dram_tensor("wqT", [P, G, D], F32R, kind="ExternalInput").ap()
    wk_ap = nc.dram_tensor("wkT", [P, G, D], F32R, kind="ExternalInput").ap()
    wv_ap = nc.dram_tensor("wvT", [P, G, D], F32R, kind="ExternalInput").ap()
    bq_ap = nc.dram_tensor("bqT", [P, G], F32, kind="ExternalInput").ap()
    bk_ap = nc.dram_tensor("bkT", [P, G], F32, kind="ExternalInput").ap()
    bv_ap = nc.dram_tensor("bvT", [P, G], F32, kind="ExternalInput").ap()
    # row of 128 ones; scaled by 1/sum(w~) it becomes the broadcast lhsT
    onesr_ap = nc.dram_tensor("onesr", [1, P], F32R, kind="ExternalInput").ap()
    out_ap = nc.dram_tensor("out", [NB, D], F32, kind="ExternalOutput").ap()

    mm = nc.tensor.matmul

    with tile.TileContext(nc) as tc, ExitStack() as ctx:
        consts = ctx.enter_context(tc.tile_pool(name="consts", bufs=1))
        pimg = ctx.enter_context(tc.tile_pool(name="pimg", bufs=2))
        pkeys = ctx.enter_context(tc.tile_pool(name="pkeys", bufs=2))
        pvals = ctx.enter_context(tc.tile_pool(name="pvals", bufs=2))
        pttr = ctx.enter_context(tc.tile_pool(name="pttr", bufs=3))
        psmall = ctx.enter_context(tc.tile_pool(name="psmall", bufs=4))
        ppmm = ctx.enter_context(tc.tile_pool(name="ppmm", bufs=3, space="PSUM"))
        ppvec = ctx.enter_context(tc.tile_pool(name="ppvec", bufs=2, space="PSUM"))
        ppbc = ctx.enter_context(tc.tile_pool(name="ppbc", bufs=2, space="PSUM"))

        # ---- constants ----
        wq = consts.tile([P, G, D], F32R, tag="wq")
        nc.sync.dma_start(out=wq, in_=wq_ap)
        wk = consts.tile([P, G, D], F32R, tag="wk")
        nc.sync.dma_start(out=wk, in_=wk_ap)
        wv = consts.tile([P, G, D], F32R, tag="wv")
        nc.sync.dma_start(out=wv, in_=wv_ap)
        bq = consts.tile([P, G], F32, tag="bq")
        nc.sync.dma_start(out=bq, in_=bq_ap)
        bk = consts.tile([P, G], F32, tag="bk")
        nc.sync.dma_start(out=bk, in_=bk_ap)
        bv = consts.tile([P, G], F32, tag="bv")
        nc.sync.dma_start(out=bv, in_=bv_ap)
        onesr = consts.tile([1, P], F32R, tag="onesr")
        nc.sync.dma_start(out=onesr, in_=onesr_ap)
        ht = consts.tile([P, G, NB], F32R, tag="ht")
        nc.sync.dma_start(out=ht, in_=ht_ap)

        # ---- queries for all local batches: qt[p, dg, b] ----
        qt = consts.tile([P, G, NB], F32R, tag="qt")
        for dg in range(G):
            pq = ppvec.tile([P, NB], F32, tag="vec")
            for cg in range(G):
                mm(pq, lhsT=wq[:, cg, ds(dg * P, P)], rhs=ht[:, cg, :],
                   start=(cg == 0), stop=(cg == G - 1))
            nc.scalar.activation(out=qt[:, dg, :], in_=pq, func=Tanh,
                                 bias=bq[:, dg:dg + 1], scale=1.0)

        # ---- per-batch pipeline ----
        for _rep in range(repeat):
            for b in range(NB):
                img = pimg.tile([P, G, N], F32R, tag="img")
                for cg in range(G):
                    nc.sync.dma_start(out=img[:, cg, :],
                                      in_=img_ap[b, ds(cg * P, P), :])

                # keysT / valsT [d, n] = tanh(W @ x + bias), fused on ScalarE
                keys = pkeys.tile([P, G, N], F32R, tag="keys")
                vals = pvals.tile([P, G, N], F32, tag="vals")
                for dg in range(G):
                    for hf in range(2):
                        pk = ppmm.tile([P, 512], F32, tag="mm")
                        for cg in range(G):
                            mm(pk, lhsT=wk[:, cg, ds(dg * P, P)],
                               rhs=img[:, cg, ds(hf * 512, 512)],
                               start=(cg == 0), stop=(cg == G - 1))
                        nc.scalar.activation(
                            out=keys[:, dg, ds(hf * 512, 512)], in_=pk,
                            func=Tanh, bias=bk[:, dg:dg + 1], scale=1.0)
                        pv = ppmm.tile([P, 512], F32, tag="mm")
                        for cg in range(G):
                            mm(pv, lhsT=wv[:, cg, ds(dg * P, P)],
                               rhs=img[:, cg, ds(hf * 512, 512)],
                               start=(cg == 0), stop=(cg == G - 1))
                        nc.scalar.activation(
                            out=vals[:, dg, ds(hf * 512, 512)], in_=pv,
                            func=Tanh, bias=bv[:, dg:dg + 1], scale=1.0)

                # scores[n] = q . keysT[:, n]; w~ = exp(scores), sum on the fly
                wexp = psmall.tile([1, N], F32R, tag="wexp")
                s01 = psmall.tile([1, 2], F32, tag="s01")
                for hf in range(2):
                    psc = ppvec.tile([1, 512], F32, tag="vec")
                    for dg in range(G):
                        mm(psc, lhsT=qt[:, dg, b:b + 1],
                           rhs=keys[:, dg, ds(hf * 512, 512)],
                           start=(dg == 0), stop=(dg == G - 1))
                    nc.scalar.activation(out=wexp[0:1, ds(hf * 512, 512)],
                                         in_=psc, func=Exp,
                                         accum_out=s01[0:1, hf:hf + 1])

                # 1/sum(w~) as a 128-wide f32r row for the broadcast matmul
                stot = psmall.tile([1, 1], F32, tag="stot")
                nc.vector.tensor_add(out=stot, in0=s01[0:1, 0:1],
                                     in1=s01[0:1, 1:2])
                rtot = psmall.tile([1, 1], F32, tag="rtot")
                nc.vector.reciprocal(out=rtot, in_=stot)
                rrow = psmall.tile([1, P], F32R, tag="rrow")
                nc.vector.tensor_scalar_mul(rrow, onesr, rtot)

                # context[d] = sum_n w[n] valsT[d, n] via broadcast + DVE
                # reduce: pb[m, n] = w~[n]/sum  for all partitions m
                # tensor_tensor_reduce needs a custom DVE table the runtime
                # can't load here; DVE multiply + ScalarE Copy-with-accum is
                # equivalent and uses only HW-proven constructs.
                ctxh = psmall.tile([P, G, 2], F32, tag="ctxh")
                for hf in range(2):
                    pb = ppbc.tile([P, 512], F32, tag="bc")
                    mm(pb, lhsT=rrow, rhs=wexp[0:1, ds(hf * 512, 512)],
                       start=True, stop=True)
                    for dg in range(G):
                        tout = pttr.tile([P, 512], F32, tag="ttr")
                        nc.vector.tensor_mul(tout,
                                             vals[:, dg, ds(hf * 512, 512)],
                                             pb)
                        nc.scalar.activation(
                            out=tout, in_=tout, func=Copy,
                            accum_out=ctxh[:, dg, hf:hf + 1])

                ctxs = psmall.tile([P, G], F32, tag="ctxs")
                nc.vector.tensor_add(out=ctxs, in0=ctxh[:, :, 0],
                                     in1=ctxh[:, :, 1])
                out_view = out_ap[b:b + 1, :].rearrange(
                    "a (g p) -> p (a g)", p=P)
                nc.sync.dma_start(out=out_view, in_=ctxs)

    nc.compile()
    return nc


def _build_bf16(repeat=1, img_internal=False):
    nc = bacc.Bacc("TRN2", target_bir_lowering=False, debug=False,
                   num_devices=NCORES)

    img_kind = "Internal" if img_internal else "ExternalInput"
    img_ap = nc.dram_tensor("img", [NB, D, N], F32, kind=img_kind).ap()
    ht_ap = nc.dram_tensor("hT", [P, G, NB], F32, kind="ExternalInput").ap()
    wq_ap = nc.dram_tensor("wqT", [P, G, D], BF, kind="ExternalInput").ap()
    wk_ap = nc.dram_tensor("wkT", [P, G, D], BF, kind="ExternalInput").ap()
    wv_ap = nc.dram_tensor("wvT", [P, G, D], BF, kind="ExternalInput").ap()
    bq_ap = nc.dram_tensor("bqT", [P, G], F32, kind="ExternalInput").ap()
    bk_ap = nc.dram_tensor("bkT", [P, G], F32, kind="ExternalInput").ap()
    bvb_ap = nc.dram_tensor("bvb", [P, D], F32, kind="ExternalInput").ap()
    out_ap = nc.dram_tensor("out", [NB, D], F32, kind="ExternalOutput").ap()

    def mm(out, lhsT, rhs, start, stop):
        nc.tensor.matmul(out, lhsT=lhsT, rhs=rhs, start=start, stop=stop)

    with tile.TileContext(nc) as tc, ExitStack() as ctx:
        consts = ctx.enter_context(tc.tile_pool(name="consts", bufs=1))
        pimg32 = ctx.enter_context(tc.tile_pool(name="pimg32", bufs=2))
        pimg16 = ctx.enter_context(tc.tile_pool(name="pimg16", bufs=2))
        pkeys = ctx.enter_context(tc.tile_pool(name="pkeys", bufs=2))
        pvals = ctx.enter_context(tc.tile_pool(name="pvals", bufs=2))
        psmall = ctx.enter_context(tc.tile_pool(name="psmall", bufs=4))
        ppmm = ctx.enter_context(tc.tile_pool(name="ppmm", bufs=3, space="PSUM"))
        ppvec = ctx.enter_context(tc.tile_pool(name="ppvec", bufs=3, space="PSUM"))
        ppt = ctx.enter_context(tc.tile_pool(name="ppt", bufs=2, space="PSUM"))

        wq = consts.tile([P, G, D], BF, tag="wq")
        nc.sync.dma_start(out=wq, in_=wq_ap)
        wk = consts.tile([P, G, D], BF, tag="wk")
        nc.sync.dma_start(out=wk, in_=wk_ap)
        wv = consts.tile([P, G, D], BF, tag="wv")
        nc.sync.dma_start(out=wv, in_=wv_ap)
        bq = consts.tile([P, G], F32, tag="bq")
        nc.sync.dma_start(out=bq, in_=bq_ap)
        bk = consts.tile([P, G], F32, tag="bk")
        nc.sync.dma_start(out=bk, in_=bk_ap)
        bvb = consts.tile([P, D], F32, tag="bvb")
        nc.sync.dma_start(out=bvb, in_=bvb_ap)
        ht32 = consts.tile([P, G, NB], F32, tag="ht32")
        nc.sync.dma_start(out=ht32, in_=ht_ap)
        ones = consts.tile([1, 1], BF, tag="ones")
        nc.vector.memset(ones, 1.0)
        ht = consts.tile([P, G, NB], BF, tag="ht")
        nc.vector.tensor_copy(out=ht, in_=ht32)

        qt = consts.tile([P, G, NB], BF, tag="qt")
        for dg in range(G):
            pq = ppt.tile([P, NB], F32, tag="t")
            for cg in range(G):
                mm(pq, wq[:, cg, ds(dg * P, P)], ht[:, cg, :],
                   start=(cg == 0), stop=(cg == G - 1))
            nc.scalar.activation(out=qt[:, dg, :], in_=pq, func=Tanh,
                                 bias=bq[:, dg:dg + 1], scale=1.0)

        for _rep in range(repeat):
            for b in range(NB):
                img32 = pimg32.tile([P, G, N], F32, tag="img32")
                for cg in range(G):
                    nc.sync.dma_start(out=img32[:, cg, :],
                                      in_=img_ap[b, ds(cg * P, P), :])
                img16 = pimg16.tile([P, G, N], BF, tag="img16")
                for cg in range(G):
                    nc.vector.tensor_copy(out=img16[:, cg, :],
                                          in_=img32[:, cg, :])

                keys = pkeys.tile([P, G, N], BF, tag="keys")
                for dg in range(G):
                    for hf in range(2):
                        pk = ppmm.tile([P, 512], F32, tag="mm")
                        for cg in range(G):
                            mm(pk, wk[:, cg, ds(dg * P, P)],
                               img16[:, cg, ds(hf * 512, 512)],
                               start=(cg == 0), stop=(cg == G - 1))
                        nc.scalar.activation(
                            out=keys[:, dg, ds(hf * 512, 512)], in_=pk,
                            func=Tanh, bias=bk[:, dg:dg + 1], scale=1.0)

                vals = pvals.tile([P, NB, D], BF, tag="vals")
                for ch in range(NB):
                    pv = ppmm.tile([P, 512], F32, tag="mm")
                    for cg in range(G):
                        mm(pv, img16[:, cg, ds(ch * P, P)], wv[:, cg, :],
                           start=(cg == 0), stop=(cg == G - 1))
                    nc.vector.tensor_add(out=pv, in0=pv, in1=bvb)
                    nc.scalar.activation(out=vals[:, ch, :], in_=pv, func=Tanh)

                wexp = psmall.tile([1, N], BF, tag="wexp")
                s01 = psmall.tile([1, 2], F32, tag="s01")
                for hf in range(2):
                    psc = ppvec.tile([1, 512], F32, tag="vec")
                    for dg in range(G):
                        mm(psc, qt[:, dg, b:b + 1],
                           keys[:, dg, ds(hf * 512, 512)],
                           start=(dg == 0), stop=(dg == G - 1))
                    nc.scalar.activation(out=wexp[0:1, ds(hf * 512, 512)],
                                         in_=psc, func=Exp,
                                         accum_out=s01[0:1, hf:hf + 1])

                wt = psmall.tile([P, NB], BF, tag="wt")
                for ch in range(NB):
                    pt = ppt.tile([P, 1], F32, tag="t")
                    mm(pt, wexp[0:1, ds(ch * P, P)], ones,
                       start=True, stop=True)
                    nc.vector.tensor_copy(out=wt[:, ch:ch + 1], in_=pt[:, 0:1])

                pc = ppvec.tile([1, D], F32, tag="vec")
                for ch in range(NB):
                    mm(pc, wt[:, ch:ch + 1], vals[:, ch, :],
                       start=(ch == 0), stop=(ch == NB - 1))

                stot = psmall.tile([1, 1], F32, tag="stot")
                nc.vector.tensor_add(out=stot, in0=s01[0:1, 0:1],
                                     in1=s01[0:1, 1:2])
                rtot = psmall.tile([1, 1], F32, tag="rtot")
                nc.vector.reciprocal(out=rtot, in_=stot)
                osb = psmall.tile([1, D], F32, tag="osb")
                nc.vector.tensor_scalar_mul(osb, pc, rtot)
                nc.sync.dma_start(out=out_ap[b:b + 1, :], in_=osb)

    nc.compile()
    return nc


def _get_nc(mode=MODE, repeat=1, img_internal=False):
    key = (mode, repeat, img_internal)
    if key not in _CACHED:
        if mode == "bf16":
            _CACHED[key] = _build_bf16(repeat, img_internal)
        elif mode == "f32rv2":
            _CACHED[key] = _build_f32r(repeat, img_internal)
        elif mode == "dr8":
            _CACHED[key] = _build_dr8(repeat, img_internal)
        else:
            _CACHED[key] = _build_f32r_v1(repeat, img_internal)
    return _CACHED[key]


def _fp8_split(x, scale):
    """Return (hi, lo) fp8 e4m3 arrays of x*scale."""
    xs = np.asarray(x, np.float32) * scale
    hi = xs.astype(ml_dtypes.float8_e4m3)
    lo = (xs - hi.astype(np.float32)).astype(ml_dtypes.float8_e4m3)
    return hi, lo


def _dr8_weight(W):
    """[512, 512] W[d, c] -> hi/lo [128, 4, 512] with w[p, g, d] scaled."""
    WT = np.ascontiguousarray(np.asarray(W, dtype=np.float32).T)  # [c, d]
    hi, lo = _fp8_split(WT, WS)
    f = lambda t: np.ascontiguousarray(
        t.reshape(G, P, D).transpose(1, 0, 2))
    return f(hi), f(lo)


def _weight_layout(W, mode):
    # [512, 512] W[d, c] -> [128, 4, 512] with w[p, g, d] = W[d, g*128+p]
    WT = np.ascontiguousarray(np.asarray(W, dtype=np.float32).T)  # [c, d]
    t = np.ascontiguousarray(WT.reshape(G, P, D).transpose(1, 0, 2))
    return t.astype(ml_dtypes.bfloat16) if mode == "bf16" else t


def _bias_layout(b):
    # [512] -> [128, 4] with out[p, g] = b[g*128 + p]
    return np.ascontiguousarray(
        np.asarray(b, dtype=np.float32).reshape(G, P).T)


def make_in_maps(channel_img, last_hidden_lstm, Wq, bq, Wk, bk, Wv, bv,
                 mode=MODE):
    channel_img = np.asarray(channel_img, dtype=np.float32)
    last_hidden_lstm = np.asarray(last_hidden_lstm, dtype=np.float32)
    B, C, H, W = channel_img.shape
    assert (B, C, H * W) == (NCORES * NB, D, N)
    img_full = channel_img.reshape(B, C, H * W)

    if mode == "dr8":
        wkh, wkl = _dr8_weight(Wk)
        wvh, wvl = _dr8_weight(Wv)
        bvb = np.ascontiguousarray(np.broadcast_to(
            np.asarray(bv, np.float32) * XS * WS, (P, D)))
        wqT = np.ascontiguousarray(
            np.asarray(Wq, np.float32).T.reshape(G, P, D).transpose(1, 0, 2))
        bqT = _bias_layout(bq)
        bkT = _bias_layout(bk)
        in_maps = []
        for i in range(NCORES):
            imgc = img_full[i * NB:(i + 1) * NB]            # [NB, 512, 1024]
            xh, xl = _fp8_split(imgc, XS)
            f = lambda t: t.reshape(NB, G, P, N).transpose(0, 2, 1, 3)
            img8 = np.ascontiguousarray(
                np.stack([f(xh), f(xl)], axis=1))           # [NB,2,P,G,N]
            h = last_hidden_lstm[i * NB:(i + 1) * NB]       # [NB, 512]
            ht = np.ascontiguousarray(
                h.T.reshape(G, P, NB).transpose(1, 0, 2))
            in_maps.append({
                "img8": img8, "wkh": wkh, "wkl": wkl,
                "wvh": wvh, "wvl": wvl, "bvb": bvb,
                "wqT": wqT, "hT": ht, "bqT": bqT, "bkT": bkT,
                "onc": np.ones((P, 1), np.float32),
                "onr": np.ones((1, P), np.float32),
            })
        return in_maps

    wqT = _weight_layout(Wq, mode)
    wkT = _weight_layout(Wk, mode)
    wvT = _weight_layout(Wv, mode)
    bqT = _bias_layout(bq)
    bkT = _bias_layout(bk)

    in_maps = []
    for i in range(NCORES):
        h = last_hidden_lstm[i * NB:(i + 1) * NB]        # [NB, 512]
        ht = np.ascontiguousarray(h.T.reshape(G, P, NB).transpose(1, 0, 2))
        m = {
            "img": np.ascontiguousarray(img_full[i * NB:(i + 1) * NB]),
            "hT": ht,
            "wqT": wqT, "wkT": wkT, "wvT": wvT,
            "bqT": bqT, "bkT": bkT,
        }
        if mode == "bf16":
            m["bvb"] = np.ascontiguousarray(
                np.broadcast_to(np.asarray(bv, dtype=np.float32), (P, D)))
        elif mode == "f32rv2":
            m["bvT"] = _bias_layout(bv)
            m["onesr"] = np.ones((1, P), np.float32)
        else:
            m["bvb"] = np.ascontiguousarray(
                np.broadcast_to(np.asarray(bv, dtype=np.float32), (P, D)))
            m["onec"] = np.array([[1.0, 0.0]], np.float32)
        in_maps.append(m)
    return in_maps


def run(in_maps, mode=MODE, repeat=1, **kwargs):
    nc = _get_nc(mode, repeat)
    res = run_bass_kernel_spmd(nc, in_maps, core_ids=list(range(NCORES)),
                               **kwargs)
    outs = []
    for i in range(NCORES):
        o = np.asarray(res.results[i]["out"])
        if mode == "dr8":                       # [NB, P, G] -> [NB, D]
            o = o.transpose(0, 2, 1).reshape(NB, D)
        outs.append(o)
    out = np.concatenate(outs, axis=0)
    return np.ascontiguousarray(out.astype(np.float32)), res


def kernel(channel_img, last_hidden_lstm, Wq, bq, Wk, bk, Wv, bv):
    in_maps = make_in_maps(channel_img, last_hidden_lstm,
                           Wq, bq, Wk, bk, Wv, bv, mode=MODE)
    out, _ = run(in_maps, mode=MODE)
    return out

